# revision 9
# baseline (speedup 1.0000x reference)
"""CrossAttention TRN2 kernel — batch x head-group tensor parallel, all-bf16, no collectives.

8 cores: core c -> batch b=c//4, head-pair hg=c%4 (heads 2hg, 2hg+1), ALL 2048
queries. Per the TP sharding hint: Wq/Wk/Wv column-sharded by head pair,
Wo row-sharded; each core emits a rank-128 PARTIAL output (bf16) with bias/4
folded in, and unshard() sums the 4 partials per batch on the host.
This removes the 4x-replicated K/V projections of the seq-chunk sharding:
per-core PE work drops from ~336k to ~229k cycles.

Per core (all matmul inputs bf16, PSUM accumulation fp32):
  load:  full x_b + full ctx_b + W*[:, hg-slice] cast bf16 via gpsimd chunk DMAs
  tpose: ctxT[p,kc,m], xT[p,kc,n] via PE transpose + DVE copy out of PSUM
  proj:  qT = Wq_hg.T@xT [128,2048]; kT = Wk_hg.T@ctxT [128,2048];
         vaug = ctx@Wv_hg ones-augmented [128,130] per m-chunk
  attn:  per (head, 512-query block): S.T = kT_h.T @ qT_h; exp on ACT -> bf16;
         AV: oX[65,512] += vaug_h.T @ expST (row 64 = denom);
         normalize: ACT den copy + DVE recip_approx + gpsimd bcast + DVE mul
  out:   partial[n,1024] = oT.T @ Wo_hg + bo/4 -> bf16 DMA out
PSUM: proj: pst 2 + pp 4 banks; attn: S 3x2 + oX 2; out: 2x1 banks.
"""
import sys
sys.path.insert(0, '/opt/trn_rl_repo')
import contextlib
import numpy as np
import concourse.bass as bass
import concourse.mybir as mybir
import concourse.tile as tile
from concourse import bacc
from concourse.masks import make_identity

F32 = mybir.dt.float32
BF16 = mybir.dt.bfloat16
AF = mybir.ActivationFunctionType

B, N, M, KDIM, H, D = 2, 2048, 2048, 1024, 8, 64
INNER = H * D          # 512
SCALE = D ** -0.5      # 0.125
KC = KDIM // 128       # 8 k-chunks
NT = N // 128          # 16 n-tiles
MC = M // 128          # 16 m-chunks
VW = 2 * (D + 1)       # 130: [vA(64) | 1 | vB(64) | 1] for this head pair
GRP = 2                # m-chunks per exp group
NB = N // 512          # 4 query blocks


def build_kernel():
    nc = bacc.Bacc("TRN2", target_bir_lowering=False, debug=False, num_devices=8)
    X = nc.dram_tensor("xc", [N, KDIM], F32, kind="ExternalInput")
    CTX = nc.dram_tensor("ctxc", [M, KDIM], F32, kind="ExternalInput")
    WQ = nc.dram_tensor("Wq", [KDIM, 128], F32, kind="ExternalInput")
    WK = nc.dram_tensor("Wk", [KDIM, 128], F32, kind="ExternalInput")
    WV = nc.dram_tensor("Wv", [KDIM, 128], F32, kind="ExternalInput")
    WO = nc.dram_tensor("Wo", [128, KDIM], F32, kind="ExternalInput")
    BO = nc.dram_tensor("bo", [1, KDIM], F32, kind="ExternalInput")
    OUT = nc.dram_tensor("outc", [N, KDIM], BF16, kind="ExternalOutput")

    with tile.TileContext(nc) as tc:
        with contextlib.ExitStack() as ctx:
            sb = ctx.enter_context(tc.tile_pool(name="sb", bufs=1))
            stage = ctx.enter_context(tc.tile_pool(name="stage", bufs=3))

            ident = sb.tile([128, 128], BF16, tag="ident")
            make_identity(nc, ident[:])

            # ---------- persistent SBUF tiles ----------
            xg = [sb.tile([128, 4 * KDIM], BF16, tag=f"xg{g}", name=f"xg{g}")
                  for g in range(4)]
            ctxn = [sb.tile([128, 4 * KDIM], BF16, tag=f"ctxn{g}", name=f"ctxn{g}")
                    for g in range(4)]
            wq = sb.tile([128, KC * 128], BF16, tag="wq")
            wk = sb.tile([128, KC * 128], BF16, tag="wk")
            wv = sb.tile([128, KC * 128], BF16, tag="wv")
            wo = sb.tile([128, KDIM], BF16, tag="wo")
            wq3 = wq[:].rearrange("p (k d) -> p k d", k=KC)
            wk3 = wk[:].rearrange("p (k d) -> p k d", k=KC)
            wv3 = wv[:].rearrange("p (k d) -> p k d", k=KC)
            ctxT = sb.tile([128, KC * M], BF16, tag="ctxT")
            ctxT3 = ctxT[:].rearrange("p (k m) -> p k m", k=KC)
            xT = sb.tile([128, KC * N], BF16, tag="xT")
            xT3 = xT[:].rearrange("p (k n) -> p k n", k=KC)
            kT0 = sb.tile([128, M], BF16, tag="kT0")
            vaug = [sb.tile([128, VW], BF16, tag=f"vg{mt}", name=f"vg{mt}")
                    for mt in range(MC)]
            qT0 = sb.tile([128, N], BF16, tag="qT0")
            oT0 = sb.tile([128, N], BF16, tag="oT0")
            bo_r = sb.tile([1, KDIM], F32, tag="bo_r")
            bo_q = sb.tile([1, KDIM], F32, tag="bo_q")
            bias_bc = sb.tile([128, KDIM], F32, tag="bias_bc")

            # ---------- input DMAs (gpsimd casting swdge), contiguous blocks ----------
            def load_w(w2, WD, nchunk, cols):
                w3 = w2[:].rearrange("p (k d) -> p k d", k=nchunk)
                for k in range(nchunk):
                    nc.gpsimd.dma_start(w3[:, k, :], WD[128 * k:128 * (k + 1), :])

            def load_rows(dst, SRC, g):
                d3 = dst[:].rearrange("p (t k) -> p t k", t=4)
                for t in range(4):
                    r0 = 512 * g + 128 * t
                    nc.gpsimd.dma_start(d3[:, t, :], SRC[r0:r0 + 128, :])

            load_rows(xg[0], X, 0)
            load_w(wq, WQ, KC, 128)
            load_w(wk, WK, KC, 128)
            load_rows(ctxn[0], CTX, 0)
            load_w(wv, WV, KC, 128)
            for g in range(1, 4):
                load_rows(xg[g], X, g)
                load_rows(ctxn[g], CTX, g)
            nc.gpsimd.dma_start(wo[:], WO[:])
            nc.sync.dma_start(bo_r[:], BO[:])
            # bias/4 so the 4 partial outputs sum to exactly one bias
            nc.scalar.activation(bo_q[:], bo_r[:], AF.Identity, bias=0.0, scale=0.25)
            nc.gpsimd.partition_broadcast(bias_bc[:], bo_q[:])

            # ones columns of vaug (constant, written once)
            for mt in range(MC):
                ones = vaug[mt][:].rearrange("p (two dd) -> p two dd", two=2)[:, :, D:D + 1]
                nc.vector.memset(ones, 1.0)

            # ---------- PE transposes + projections ----------
            def transpose_tile(pst, dstT3, src2, t):
                for kg in range(KC // 4):
                    p = pst.tile([128, 512], BF16, tag="ptr")
                    for i in range(4):
                        k = 4 * kg + i
                        nc.tensor.transpose(p[:, 128 * i:128 * (i + 1)],
                                            src2[:, 128 * k:128 * (k + 1)], ident[:])
                    dst = dstT3[:, 4 * kg:4 * (kg + 1), 128 * t:128 * (t + 1)]
                    nc.vector.tensor_copy(dst, p[:].rearrange("p (i c) -> p i c", i=4))

            with (tc.tile_pool(name="pst", bufs=2, space="PSUM") as pst,
                  tc.tile_pool(name="pp", bufs=4, space="PSUM") as pp):
                for g in range(4):
                    x3 = xg[g][:].rearrange("p (t k) -> p t k", t=4)
                    for t in range(4):
                        transpose_tile(pst, xT3, x3[:, t, :], 4 * g + t)
                    # qT for this 512-query block (needs all k of these n-cols)
                    p = pp.tile([128, 512], F32, tag="pp")
                    for k in range(KC):
                        nc.tensor.matmul(p[:], wq3[:, k, :],
                                         xT3[:, k, 512 * g:512 * (g + 1)],
                                         start=(k == 0), stop=(k == KC - 1))
                    nc.scalar.copy(qT0[:, 512 * g:512 * (g + 1)], p[:])

                    c3 = ctxn[g][:].rearrange("p (t k) -> p t k", t=4)
                    for t in range(4):
                        transpose_tile(pst, ctxT3, c3[:, t, :], 4 * g + t)
                    p = pp.tile([128, 512], F32, tag="pp")
                    for k in range(KC):
                        nc.tensor.matmul(p[:], wk3[:, k, :],
                                         ctxT3[:, k, 512 * g:512 * (g + 1)],
                                         start=(k == 0), stop=(k == KC - 1))
                    nc.scalar.copy(kT0[:, 512 * g:512 * (g + 1)], p[:])
                    for t in range(4):
                        mt = 4 * g + t
                        p = pp.tile([128, 512], F32, tag="pp")
                        for k in range(KC):
                            nc.tensor.matmul(p[:, 0:128], ctxT3[:, k, 128 * mt:128 * (mt + 1)],
                                             wv3[:, k, :], start=(k == 0), stop=(k == KC - 1))
                        pv = p[:, 0:128].rearrange("p (two d) -> p two d", two=2)
                        tv = vaug[mt][:].rearrange("p (two dd) -> p two dd", two=2)[:, :, 0:D]
                        nc.vector.tensor_copy(tv, pv)

            # ---------- attention: per (head, 512-query block) ----------
            NG = MC // GRP  # 8 groups of 2 m-chunks
            with (tc.tile_pool(name="psS", bufs=3, space="PSUM") as psS,
                  tc.tile_pool(name="psO", bufs=2, space="PSUM") as psO,
                  tc.tile_pool(name="se", bufs=3) as se):
                for head in range(2):
                    bk = 64 * head
                    vb = (D + 1) * head
                    for nb in range(NB):
                        qsl = qT0[bk:bk + 64, 512 * nb:512 * (nb + 1)]
                        oX = psO.tile([128, 512], F32, tag="oX")

                        def do_av(pend):
                            g_p, e_p = pend
                            for i in range(GRP):
                                mc = GRP * g_p + i
                                v = vaug[mc][:, vb:vb + D + 1]
                                nc.tensor.matmul(oX[0:D + 1, :], v,
                                                 e_p[:, 512 * i:512 * (i + 1)],
                                                 start=(mc == 0), stop=(mc == MC - 1))

                        pending = None
                        for g in range(NG):
                            s = psS.tile([128, 512 * GRP], F32, tag="s")
                            for i in range(GRP):
                                mc = GRP * g + i
                                ksl = kT0[bk:bk + 64, 128 * mc:128 * (mc + 1)]
                                nc.tensor.matmul(s[:, 512 * i:512 * (i + 1)], ksl,
                                                 qsl, start=True, stop=True)
                            if pending is not None:
                                do_av(pending)
                            e = se.tile([128, 512 * GRP], BF16, tag="e")
                            nc.scalar.activation(e[:], s[:], AF.Exp, bias=0.0, scale=SCALE)
                            pending = (g, e)
                        do_av(pending)
                        den = stage.tile([1, 512], F32, tag="den")
                        nc.scalar.copy(den[:], oX[D:D + 1, :])
                        rec = stage.tile([1, 512], F32, tag="rec")
                        nc.vector.reciprocal_approx_fast(rec[:], den[:])
                        rec_b = stage.tile([D, 512], F32, tag="rec_b")
                        nc.gpsimd.partition_broadcast(rec_b[:], rec[:])
                        nc.vector.tensor_mul(oT0[bk:bk + D, 512 * nb:512 * (nb + 1)],
                                             oX[0:D, :], rec_b[:])

            # ---------- partial O projection (rank-128) + bias/4, bf16 out ----------
            with (tc.tile_pool(name="psD", bufs=2, space="PSUM") as psD,
                  tc.tile_pool(name="so", bufs=3) as so):
                for nt in range(NT):
                    for hf in range(2):
                        p = psD.tile([128, 512], F32, tag="pout")
                        nc.tensor.matmul(p[:], oT0[:, 128 * nt:128 * (nt + 1)],
                                         wo[:, 512 * hf:512 * (hf + 1)],
                                         start=True, stop=True)
                        osb = so.tile([128, 512], BF16, tag="osb")
                        nc.vector.tensor_add(osb[:], p[:], bias_bc[:, 512 * hf:512 * (hf + 1)])
                        nc.sync.dma_start(
                            OUT[128 * nt:128 * (nt + 1), 512 * hf:512 * (hf + 1)], osb[:])
    nc.compile()
    return nc


def shard_inputs(inputs):
    """full inputs dict -> list of 8 per-core in_maps (batch x head-pair TP)"""
    x, ctx = np.asarray(inputs["x"]), np.asarray(inputs["context"])
    Wq, Wk = np.asarray(inputs["Wq"]), np.asarray(inputs["Wk"])
    Wv, Wo = np.asarray(inputs["Wv"]), np.asarray(inputs["Wo"])
    maps = []
    for c in range(8):
        b, hg = c // 4, c % 4
        cs = slice(128 * hg, 128 * (hg + 1))
        maps.append({
            "xc": np.ascontiguousarray(x[b]),
            "ctxc": np.ascontiguousarray(ctx[b]),
            "Wq": np.ascontiguousarray(Wq[:, cs]),
            "Wk": np.ascontiguousarray(Wk[:, cs]),
            "Wv": np.ascontiguousarray(Wv[:, cs]),
            "Wo": np.ascontiguousarray(Wo[cs, :]),
            "bo": np.asarray(inputs["bo"]).reshape(1, KDIM),
        })
    return maps


def unshard_outputs(results):
    out = np.empty((B, N, KDIM), dtype=np.float32)
    for b in range(B):
        acc = np.zeros((N, KDIM), dtype=np.float32)
        for hg in range(4):
            acc += np.asarray(results[4 * b + hg]["outc"], dtype=np.float32)
        out[b] = acc
    return out


_CACHED = {}


def kernel(**inputs):
    """Full unsharded inputs -> full output [2, 2048, 1024] fp32. Runs on 8 NeuronCores."""
    from concourse.bass_utils import run_bass_kernel_spmd
    if "nc" not in _CACHED:
        _CACHED["nc"] = build_kernel()
    nc = _CACHED["nc"]
    maps = shard_inputs(inputs)
    res = run_bass_kernel_spmd(nc, maps, list(range(8)))
    return unshard_outputs(res.results)
